# revision 12
# baseline (speedup 1.0000x reference)
"""AetherLinearPTQ Trainium2 kernel.

Computes out = float_to_af(x) @ weight.T + bias  for
x (8, 2048, 4096) f32, weight (4096, 4096) f32 (already AF16-valued),
bias (4096,) f32 (already AF16-valued).

Key observations:
  * AF16 quantization (base-4 exponent, 8-bit significand) == round the
    fp32 value to the nearest multiple of 2**(2*e_true - 6).  This is done
    exactly with the classic "add/subtract a large power-of-two S" trick,
    where S = 2**(2*e_true + 17) is built per-element from the exponent
    bits:  S0_bits = bits(f32(u & 0xFF800000) * 2) & 0xFF000000,
    S = bitcast_f32(S0_bits) * 65536.  fp32 RNE addition of S then performs
    round-half-even at exactly the right bit position, including the
    mantissa-overflow carry (matches the torch/jax reference bit-for-bit;
    verified exhaustively against float_to_af on 1M+ random values).
    Only |x| < 2**-124 (AF16-subnormal, probability ~0 for randn data)
    would round differently.
  * AF16 values have <= 8 significand bits, so both x_q and weight are
    *exactly* representable in bf16 -> the matmul runs on the PE at bf16
    rate (1 cycle/row) with fp32 PSUM accumulation.

Sharding: data parallel over the batch dim (8 batches -> 8 cores),
weight/bias replicated.  Each core computes a (2048, 4096) output.

Per-core layout: x is fed transposed (d-major) so quantized tiles are
directly usable as the stationary matmul operand; out = lhsT.T @ rhs with
lhsT = x_qT[d,s] tile, rhs = wT[d,o] tile, giving out[s,o] in natural
row-major layout (no output transpose needed).
"""

import sys

import numpy as np

if "/opt/trn_rl_repo" not in sys.path:
    sys.path.insert(0, "/opt/trn_rl_repo")

B, S, D, O = 8, 2048, 4096, 4096
N_CORES = 8
# Route the single-source quant bit-ops to GpSimd so the DVE only runs
# single-port scalar_tensor_tensor ops (no shared-port contention) and the
# per-column quant latency drops ~35%.
QUANT_S_ON_GPSIMD = False  # TensorScalarPtr not codegen-supported on Pool engine


def build_nc(d=D, s=S, o=O, ch=512, ow=2048):
    """Build the per-core Bass program.

    d: contraction dim (x features), s: tokens per core, o: out features,
    ch: quantization chunk width (elements of the flattened (k s) column),
    ow: output-feature width kept resident in SBUF at a time.
    """
    import concourse.bacc as bacc
    import concourse.mybir as mybir
    import concourse.tile as tile

    dt = mybir.dt
    Alu = mybir.AluOpType

    kt = d // 128        # contraction tiles
    ms_n = s // 128      # token blocks
    oh_n = o // ow       # weight-resident passes
    nj = ow // 512       # 512-wide matmuls per k per pass
    colw = kt * 128      # flattened quant-column width (== d)
    qc_n = colw // ch    # quant chunks per token block

    nc = bacc.Bacc("TRN2", target_bir_lowering=False, debug=False)

    xT = nc.dram_tensor("xT", [d, s], dt.float32, kind="ExternalInput")
    w = nc.dram_tensor("w", [d, o], dt.bfloat16, kind="ExternalInput")
    bb = nc.dram_tensor("bb", [128, o], dt.bfloat16, kind="ExternalInput")
    out = nc.dram_tensor("out", [s, o], dt.float32, kind="ExternalOutput")

    with tile.TileContext(nc) as tc:
        with (
            tc.tile_pool(name="const", bufs=1) as constp,
            tc.tile_pool(name="wt", bufs=1) as wtp,
            tc.tile_pool(name="xq", bufs=3) as xqp,
            tc.tile_pool(name="xf", bufs=4) as xfp,
            tc.tile_pool(name="vv", bufs=4) as vp,
            tc.tile_pool(name="m2", bufs=2) as m2p,
            tc.tile_pool(name="bt", bufs=2) as btp,
            tc.tile_pool(name="osb", bufs=2 * nj) as osbp,
            tc.tile_pool(name="ps", bufs=2 * nj, space="PSUM") as psp,
        ):
            biasb = constp.tile([128, o], dt.bfloat16)

            def emit_quant(ms):
                """Quantize the ms-th 128-token column (all of d) into a
                fresh xq tile; returns the tile."""
                xq = xqp.tile([128, colw], dt.bfloat16, tag="xq", name="xq")
                xsrc = xT[:, ms * 128 : (ms + 1) * 128].rearrange(
                    "(k p) t -> p k t", p=128
                )
                kpc = ch // 128  # k-slabs per quant chunk
                for q in range(qc_n):
                    xf = xfp.tile([128, ch], dt.float32, tag="xf", name="xf")
                    nc.sync.dma_start(
                        xf[:].rearrange("p (k t) -> p k t", t=128),
                        xsrc[:, q * kpc : (q + 1) * kpc, :],
                    )
                    # S0_bits = bits(f32(u & 0xFF800000) * 2) & 0xFF000000
                    # (walrus forbids mixing arith+bitwise in one op, and
                    # int arithmetic may run through the fp32 datapath, so
                    # keep each instruction in a single domain)
                    qeng = nc.gpsimd if QUANT_S_ON_GPSIMD else nc.vector
                    a = vp.tile([128, ch], dt.int32, tag="vv", name="a")
                    qeng.tensor_scalar(
                        a[:], xf[:].bitcast(dt.int32), -8388608, None,
                        Alu.bitwise_and,
                    )
                    m2 = m2p.tile([128, ch], dt.float32, tag="m2", name="m2")
                    qeng.tensor_scalar(
                        m2[:], a[:].bitcast(dt.float32), 2.0, None, Alu.mult
                    )
                    v = vp.tile([128, ch], dt.int32, tag="vv", name="v")
                    qeng.tensor_scalar(
                        v[:], m2[:].bitcast(dt.int32), -16777216, None,
                        Alu.bitwise_and,
                    )
                    bt = btp.tile([128, ch], dt.float32, tag="bt", name="bt")
                    # b = S0*65536 + x   (S = S0*65536 is the rounding magnet)
                    nc.vector.scalar_tensor_tensor(
                        bt[:], v[:].bitcast(dt.float32), 65536.0, xf[:],
                        Alu.mult, Alu.add,
                    )
                    # xq = b - S  (exact, <=8 significand bits -> bf16 exact)
                    nc.vector.scalar_tensor_tensor(
                        xq[:, q * ch : (q + 1) * ch],
                        v[:].bitcast(dt.float32), -65536.0, bt[:],
                        Alu.mult, Alu.add,
                    )
                return xq

            def emit_wt(oh, k):
                wtile = wtp.tile(
                    [128, ow], dt.bfloat16, tag=f"w{k}", name=f"wtile{k}"
                )
                nc.sync.dma_start(
                    wtile[:], w[k * 128 : (k + 1) * 128, oh * ow : (oh + 1) * ow]
                )
                return wtile

            for oh in range(oh_n):
                # First two weight slabs, then the first token column's
                # quantization, then the bulk weight DMAs: the PE needs
                # (wt[0], xq chunk 0) as early as possible and must not sit
                # behind 16 MB of weight transfers.
                npre = min(1, kt)
                wt = [emit_wt(oh, k) for k in range(npre)]
                pending = [emit_quant(0)]
                if oh == 0:
                    nc.sync.dma_start(biasb[:], bb[:])
                if ms_n > 1:
                    pending.append(emit_quant(1))
                for k in range(npre, kt):
                    wt.append(emit_wt(oh, k))

                for ms in range(ms_n):
                    xq = pending.pop(0)
                    if ms + 2 < ms_n:
                        pending.append(emit_quant(ms + 2))

                    # ---- matmul: out[s,o] block = x_qT.T @ wT ----
                    ps = [
                        psp.tile([128, 512], dt.float32, tag="ps", name="ps") for _ in range(nj)
                    ]
                    for k in range(kt):
                        lhsT = xq[:, k * 128 : (k + 1) * 128]
                        for j in range(nj):
                            nc.tensor.matmul(
                                ps[j][:],
                                lhsT,
                                wt[k][:, j * 512 : (j + 1) * 512],
                                start=(k == 0),
                                stop=(k == kt - 1),
                            )
                    for j in range(nj):
                        osb = osbp.tile([128, 512], dt.float32, tag="osb")
                        nc.vector.tensor_add(
                            osb[:],
                            ps[j][:],
                            biasb[:, oh * ow + j * 512 : oh * ow + (j + 1) * 512],
                        )
                        nc.sync.dma_start(
                            out[
                                ms * 128 : (ms + 1) * 128,
                                oh * ow + j * 512 : oh * ow + (j + 1) * 512,
                            ],
                            osb[:],
                        )
    nc.compile()
    _dedupe_ldweights(nc)
    return nc


def _dedupe_ldweights(nc):
    """Drop redundant InstLdweights: walrus emits LDWEIGHTS before every
    MATMUL, but consecutive matmuls sharing the same stationary operand only
    need the first load (the PE weight buffer persists across matmuls).
    Only sync-free LDWs with a byte-identical weights access pattern to the
    previous (surviving) LDW are dropped; any other instruction class in
    between resets the tracked state."""

    def sig(ins):
        ap = ins.ins[-1]
        try:
            return (
                ap.memref if hasattr(ap, "memref") else None,
                ap.offset,
                str(ap.ap),
                str(getattr(ap, "dtype", None)),
            )
        except Exception:
            return None

    dropped = 0
    for fn in nc.m.functions:
        for blk in fn.blocks:
            last = None
            keep = []
            for ins in blk.instructions:
                nm = type(ins).__name__
                if nm == "InstLdweights":
                    si = ins.sync_info
                    clean = si is None or (
                        len(si.on_wait) == 0 and len(si.on_update) == 0
                    )
                    s = sig(ins)
                    if clean and s is not None and s == last:
                        dropped += 1
                        continue  # redundant reload of identical weights
                    last = s if s is not None else None
                    keep.append(ins)
                elif nm in ("InstMatmult", "InstEventSemaphore", "InstNop"):
                    keep.append(ins)
                else:
                    last = None
                    keep.append(ins)
            if len(keep) != len(blk.instructions):
                blk.instructions[:] = keep
    return dropped


def _prep_inputs(x, weight, bias):
    import ml_dtypes

    x = np.ascontiguousarray(np.asarray(x, dtype=np.float32))
    weight = np.asarray(weight, dtype=np.float32)
    bias = np.asarray(bias, dtype=np.float32)

    # per-core transposed x: (B, D, S)
    xT = np.ascontiguousarray(x.transpose(0, 2, 1))
    # weight.T in bf16 (exact: AF16 subset of bf16), contiguous (D, O)
    wT = np.ascontiguousarray(weight.T).astype(ml_dtypes.bfloat16)
    bb = np.ascontiguousarray(
        np.broadcast_to(bias.reshape(1, -1), (128, bias.shape[0]))
    ).astype(ml_dtypes.bfloat16)
    return xT, wT, bb


def kernel(x, weight, bias, _trace=False, _trace_kwargs=None):
    from concourse import bass_utils

    xT, wT, bb = _prep_inputs(x, weight, bias)
    nc = build_nc()
    in_maps = [{"xT": xT[c], "w": wT, "bb": bb} for c in range(N_CORES)]
    res = bass_utils.run_bass_kernel_spmd(
        nc,
        in_maps,
        core_ids=list(range(N_CORES)),
        trace=_trace,
        **(_trace_kwargs or {}),
    )
    outs = np.stack([res.results[c]["out"] for c in range(N_CORES)])
    if _trace:
        kernel.last_results = res
    return outs.astype(np.float32, copy=False)


if __name__ == "__main__":
    rng = np.random.default_rng(0)
    x = rng.standard_normal((B, S, D), dtype=np.float32)
    weight = rng.standard_normal((O, D), dtype=np.float32) * 0.02
    bias = rng.standard_normal((O,), dtype=np.float32) * 0.02
    out = kernel(x, weight, bias)
    print(out.shape, out.dtype)


# revision 14
# speedup vs baseline: 1.0008x; 1.0008x over previous
"""AetherLinearPTQ Trainium2 kernel.

Computes out = float_to_af(x) @ weight.T + bias  for
x (8, 2048, 4096) f32, weight (4096, 4096) f32 (already AF16-valued),
bias (4096,) f32 (already AF16-valued).

Key observations:
  * AF16 quantization (base-4 exponent, 8-bit significand) == round the
    fp32 value to the nearest multiple of 2**(2*e_true - 6).  This is done
    exactly with the classic "add/subtract a large power-of-two S" trick,
    where S = 2**(2*e_true + 17) is built per-element from the exponent
    bits:  S0_bits = bits(f32(u & 0xFF800000) * 2) & 0xFF000000,
    S = bitcast_f32(S0_bits) * 65536.  fp32 RNE addition of S then performs
    round-half-even at exactly the right bit position, including the
    mantissa-overflow carry (matches the torch/jax reference bit-for-bit;
    verified exhaustively against float_to_af on 1M+ random values).
    Only |x| < 2**-124 (AF16-subnormal, probability ~0 for randn data)
    would round differently.
  * AF16 values have <= 8 significand bits, so both x_q and weight are
    *exactly* representable in bf16 -> the matmul runs on the PE at bf16
    rate (1 cycle/row) with fp32 PSUM accumulation.

Sharding: data parallel over the batch dim (8 batches -> 8 cores),
weight/bias replicated.  Each core computes a (2048, 4096) output.

Per-core layout: x is fed transposed (d-major) so quantized tiles are
directly usable as the stationary matmul operand; out = lhsT.T @ rhs with
lhsT = x_qT[d,s] tile, rhs = wT[d,o] tile, giving out[s,o] in natural
row-major layout (no output transpose needed).
"""

import sys

import numpy as np

if "/opt/trn_rl_repo" not in sys.path:
    sys.path.insert(0, "/opt/trn_rl_repo")

B, S, D, O = 8, 2048, 4096, 4096
N_CORES = 8
# Route the single-source quant bit-ops to GpSimd so the DVE only runs
# single-port scalar_tensor_tensor ops (no shared-port contention) and the
# per-column quant latency drops ~35%.
QUANT_S_ON_GPSIMD = False  # TensorScalarPtr not codegen-supported on Pool engine


def build_nc(d=D, s=S, o=O, ch=512, ow=2048):
    """Build the per-core Bass program.

    d: contraction dim (x features), s: tokens per core, o: out features,
    ch: quantization chunk width (elements of the flattened (k s) column),
    ow: output-feature width kept resident in SBUF at a time.
    """
    import concourse.bacc as bacc
    import concourse.mybir as mybir
    import concourse.tile as tile

    dt = mybir.dt
    Alu = mybir.AluOpType

    kt = d // 128        # contraction tiles
    ms_n = s // 128      # token blocks
    oh_n = o // ow       # weight-resident passes
    nj = ow // 512       # 512-wide matmuls per k per pass
    colw = kt * 128      # flattened quant-column width (== d)
    qc_n = colw // ch    # quant chunks per token block

    nc = bacc.Bacc("TRN2", target_bir_lowering=False, debug=False)

    xT = nc.dram_tensor("xT", [d, s], dt.float32, kind="ExternalInput")
    w = nc.dram_tensor("w", [d, o], dt.bfloat16, kind="ExternalInput")
    bb = nc.dram_tensor("bb", [128, o], dt.bfloat16, kind="ExternalInput")
    out = nc.dram_tensor("out", [s, o], dt.float32, kind="ExternalOutput")

    with tile.TileContext(nc) as tc:
        with (
            tc.tile_pool(name="const", bufs=1) as constp,
            tc.tile_pool(name="wt", bufs=1) as wtp,
            tc.tile_pool(name="xq", bufs=3) as xqp,
            tc.tile_pool(name="xf", bufs=4) as xfp,
            tc.tile_pool(name="vv", bufs=4) as vp,
            tc.tile_pool(name="m2", bufs=2) as m2p,
            tc.tile_pool(name="bt", bufs=2) as btp,
            tc.tile_pool(name="osb", bufs=2 * nj) as osbp,
            tc.tile_pool(name="ps", bufs=2 * nj, space="PSUM") as psp,
        ):
            biasb = constp.tile([128, o], dt.bfloat16)

            def emit_quant(ms):
                """Quantize the ms-th 128-token column (all of d) into a
                fresh xq tile; returns the tile."""
                xq = xqp.tile([128, colw], dt.bfloat16, tag="xq", name="xq")
                xsrc = xT[:, ms * 128 : (ms + 1) * 128].rearrange(
                    "(k p) t -> p k t", p=128
                )
                kpc = ch // 128  # k-slabs per quant chunk
                for q in range(qc_n):
                    xf = xfp.tile([128, ch], dt.float32, tag="xf", name="xf")
                    nc.sync.dma_start(
                        xf[:].rearrange("p (k t) -> p k t", t=128),
                        xsrc[:, q * kpc : (q + 1) * kpc, :],
                    )
                    # S0_bits = bits(f32(u & 0xFF800000) * 2) & 0xFF000000
                    # (walrus forbids mixing arith+bitwise in one op, and
                    # int arithmetic may run through the fp32 datapath, so
                    # keep each instruction in a single domain)
                    qeng = nc.gpsimd if QUANT_S_ON_GPSIMD else nc.vector
                    a = vp.tile([128, ch], dt.int32, tag="vv", name="a")
                    qeng.tensor_scalar(
                        a[:], xf[:].bitcast(dt.int32), -8388608, None,
                        Alu.bitwise_and,
                    )
                    m2 = m2p.tile([128, ch], dt.float32, tag="m2", name="m2")
                    qeng.tensor_scalar(
                        m2[:], a[:].bitcast(dt.float32), 2.0, None, Alu.mult
                    )
                    v = vp.tile([128, ch], dt.int32, tag="vv", name="v")
                    qeng.tensor_scalar(
                        v[:], m2[:].bitcast(dt.int32), -16777216, None,
                        Alu.bitwise_and,
                    )
                    bt = btp.tile([128, ch], dt.float32, tag="bt", name="bt")
                    # b = S0*65536 + x   (S = S0*65536 is the rounding magnet)
                    nc.vector.scalar_tensor_tensor(
                        bt[:], v[:].bitcast(dt.float32), 65536.0, xf[:],
                        Alu.mult, Alu.add,
                    )
                    # xq = b - S  (exact, <=8 significand bits -> bf16 exact)
                    nc.vector.scalar_tensor_tensor(
                        xq[:, q * ch : (q + 1) * ch],
                        v[:].bitcast(dt.float32), -65536.0, bt[:],
                        Alu.mult, Alu.add,
                    )
                return xq

            def emit_wt(oh, k, half=None):
                """DMA one weight slab.  half=None: whole [128, ow] slab in
                one transfer (allocates the tile).  half=(tile, 0|1): fill
                only that half of an existing tile (used to defer the high
                half of pass 0 past the ramp; MMs on each half depend only
                on its own DMA)."""
                if half is None:
                    wtile = wtp.tile(
                        [128, ow], dt.bfloat16, tag=f"w{k}", name=f"wtile{k}"
                    )
                    nc.sync.dma_start(
                        wtile[:],
                        w[k * 128 : (k + 1) * 128, oh * ow : (oh + 1) * ow],
                    )
                    return wtile
                wtile, h = half
                hw_ = ow // 2
                nc.sync.dma_start(
                    wtile[:, h * hw_ : (h + 1) * hw_],
                    w[
                        k * 128 : (k + 1) * 128,
                        oh * ow + h * hw_ : oh * ow + (h + 1) * hw_,
                    ],
                )
                return wtile

            def emit_group(oh, ms, js, xq, wt):
                """One accumulation group: output block out[ms, oh*ow + js
                512-columns], full contraction over kt k-slabs."""
                ps = [
                    psp.tile([128, 512], dt.float32, tag="ps", name="ps")
                    for _ in js
                ]
                for k in range(kt):
                    lhsT = xq[:, k * 128 : (k + 1) * 128]
                    for i, j in enumerate(js):
                        nc.tensor.matmul(
                            ps[i][:],
                            lhsT,
                            wt[k][:, j * 512 : (j + 1) * 512],
                            start=(k == 0),
                            stop=(k == kt - 1),
                        )
                for i, j in enumerate(js):
                    osb = osbp.tile([128, 512], dt.float32, tag="osb")
                    nc.vector.tensor_add(
                        osb[:],
                        ps[i][:],
                        biasb[:, oh * ow + j * 512 : oh * ow + (j + 1) * 512],
                    )
                    nc.sync.dma_start(
                        out[
                            ms * 128 : (ms + 1) * 128,
                            oh * ow + j * 512 : oh * ow + (j + 1) * 512,
                        ],
                        osb[:],
                    )

            all_js = tuple(range(nj))
            lo_js = tuple(range(nj // 2)) or all_js
            hi_js = tuple(range(nj // 2, nj))
            for oh in range(oh_n):
                split_ramp = oh == 0 and ms_n > 1 and nj >= 2
                if split_ramp:
                    # Ramp is DMA-bound on the first column's weights: feed
                    # the PE the low output-half (half the weight bytes)
                    # for columns 0-1 first; the high half streams during
                    # that compute.
                    wt = [
                        wtp.tile(
                            [128, ow], dt.bfloat16, tag=f"w{k}", name=f"wtile{k}"
                        )
                        for k in range(kt)
                    ]
                    emit_wt(oh, 0, (wt[0], 0))
                    xq0 = emit_quant(0)
                    nc.sync.dma_start(biasb[:], bb[:])
                    xq1 = emit_quant(1)
                    for k in range(1, kt):
                        emit_wt(oh, k, (wt[k], 0))
                    for k in range(kt):
                        emit_wt(oh, k, (wt[k], 1))
                    groups = (
                        [(0, lo_js), (1, lo_js), (0, hi_js), (1, hi_js)]
                        + [(m, all_js) for m in range(2, ms_n)]
                    )
                    col_tiles = {0: xq0, 1: xq1}
                else:
                    wt = [emit_wt(oh, 0)]
                    xq0 = emit_quant(0)
                    if oh == 0:
                        nc.sync.dma_start(biasb[:], bb[:])
                    col_tiles = {0: xq0}
                    if ms_n > 1:
                        col_tiles[1] = emit_quant(1)
                    for k in range(1, kt):
                        wt.append(emit_wt(oh, k))
                    groups = [(m, all_js) for m in range(ms_n)]

                # columns quantized two ahead of their first (full) use;
                # during the split ramp, hold prefetch back so the weight
                # high-half DMAs aren't stuck behind extra x chunks
                for idx, (ms, js) in enumerate(groups):
                    if ms not in col_tiles:
                        col_tiles[ms] = emit_quant(ms)
                    if not (split_ramp and idx < 2):
                        nxt = max(col_tiles) + 1
                        if nxt < ms_n and nxt <= ms + 2:
                            col_tiles[nxt] = emit_quant(nxt)
                    emit_group(oh, ms, js, col_tiles[ms], wt)
    nc.compile()
    _dedupe_ldweights(nc)
    return nc


def _dedupe_ldweights(nc):
    """Drop redundant InstLdweights: walrus emits LDWEIGHTS before every
    MATMUL, but consecutive matmuls sharing the same stationary operand only
    need the first load (the PE weight buffer persists across matmuls).
    Only sync-free LDWs with a byte-identical weights access pattern to the
    previous (surviving) LDW are dropped; any other instruction class in
    between resets the tracked state."""

    def sig(ins):
        ap = ins.ins[-1]
        try:
            return (
                ap.memref if hasattr(ap, "memref") else None,
                ap.offset,
                str(ap.ap),
                str(getattr(ap, "dtype", None)),
            )
        except Exception:
            return None

    dropped = 0
    for fn in nc.m.functions:
        for blk in fn.blocks:
            last = None
            keep = []
            for ins in blk.instructions:
                nm = type(ins).__name__
                if nm == "InstLdweights":
                    si = ins.sync_info
                    clean = si is None or (
                        len(si.on_wait) == 0 and len(si.on_update) == 0
                    )
                    s = sig(ins)
                    if clean and s is not None and s == last:
                        dropped += 1
                        continue  # redundant reload of identical weights
                    last = s if s is not None else None
                    keep.append(ins)
                elif nm in ("InstMatmult", "InstEventSemaphore", "InstNop"):
                    keep.append(ins)
                else:
                    last = None
                    keep.append(ins)
            if len(keep) != len(blk.instructions):
                blk.instructions[:] = keep
    return dropped


def _prep_inputs(x, weight, bias):
    import ml_dtypes

    x = np.ascontiguousarray(np.asarray(x, dtype=np.float32))
    weight = np.asarray(weight, dtype=np.float32)
    bias = np.asarray(bias, dtype=np.float32)

    # per-core transposed x: (B, D, S)
    xT = np.ascontiguousarray(x.transpose(0, 2, 1))
    # weight.T in bf16 (exact: AF16 subset of bf16), contiguous (D, O)
    wT = np.ascontiguousarray(weight.T).astype(ml_dtypes.bfloat16)
    bb = np.ascontiguousarray(
        np.broadcast_to(bias.reshape(1, -1), (128, bias.shape[0]))
    ).astype(ml_dtypes.bfloat16)
    return xT, wT, bb


def kernel(x, weight, bias, _trace=False, _trace_kwargs=None):
    from concourse import bass_utils

    xT, wT, bb = _prep_inputs(x, weight, bias)
    nc = build_nc()
    in_maps = [{"xT": xT[c], "w": wT, "bb": bb} for c in range(N_CORES)]
    res = bass_utils.run_bass_kernel_spmd(
        nc,
        in_maps,
        core_ids=list(range(N_CORES)),
        trace=_trace,
        **(_trace_kwargs or {}),
    )
    outs = np.stack([res.results[c]["out"] for c in range(N_CORES)])
    if _trace:
        kernel.last_results = res
    return outs.astype(np.float32, copy=False)


if __name__ == "__main__":
    rng = np.random.default_rng(0)
    x = rng.standard_normal((B, S, D), dtype=np.float32)
    weight = rng.standard_normal((O, D), dtype=np.float32) * 0.02
    bias = rng.standard_normal((O,), dtype=np.float32) * 0.02
    out = kernel(x, weight, bias)
    print(out.shape, out.dtype)


# revision 15
# speedup vs baseline: 1.1849x; 1.1840x over previous
"""AetherLinearPTQ Trainium2 kernel.

Computes out = float_to_af(x) @ weight.T + bias  for
x (8, 2048, 4096) f32, weight (4096, 4096) f32 (already AF16-valued),
bias (4096,) f32 (already AF16-valued).

Key observations:
  * AF16 quantization (base-4 exponent, 8-bit significand) == round the
    fp32 value to the nearest multiple of 2**(2*e_true - 6).  This is done
    exactly with the classic "add/subtract a large power-of-two S" trick,
    where S = 2**(2*e_true + 17) is built per-element from the exponent
    bits:  S0_bits = bits(f32(u & 0xFF800000) * 2) & 0xFF000000,
    S = bitcast_f32(S0_bits) * 65536.  fp32 RNE addition of S then performs
    round-half-even at exactly the right bit position, including the
    mantissa-overflow carry (matches the torch/jax reference bit-for-bit;
    verified exhaustively against float_to_af on 1M+ random values).
    Only |x| < 2**-124 (AF16-subnormal, probability ~0 for randn data)
    would round differently.
  * AF16 values have <= 8 significand bits, so both x_q and weight are
    *exactly* representable in bf16 -> the matmul runs on the PE at bf16
    rate (1 cycle/row) with fp32 PSUM accumulation.

Sharding: data parallel over the batch dim (8 batches -> 8 cores),
weight/bias replicated.  Each core computes a (2048, 4096) output.

Per-core layout: x is fed transposed (d-major) so quantized tiles are
directly usable as the stationary matmul operand; out = lhsT.T @ rhs with
lhsT = x_qT[d,s] tile, rhs = wT[d,o] tile, giving out[s,o] in natural
row-major layout (no output transpose needed).
"""

import sys

import numpy as np

if "/opt/trn_rl_repo" not in sys.path:
    sys.path.insert(0, "/opt/trn_rl_repo")

B, S, D, O = 8, 2048, 4096, 4096
N_CORES = 8
# Route the single-source quant bit-ops to GpSimd so the DVE only runs
# single-port scalar_tensor_tensor ops (no shared-port contention) and the
# per-column quant latency drops ~35%.
QUANT_S_ON_GPSIMD = False  # TensorScalarPtr not codegen-supported on Pool engine


def build_nc(d=D, s=S, o=O, ch=512, ow=2048):
    """Build the per-core Bass program.

    d: contraction dim (x features), s: tokens per core, o: out features,
    ch: quantization chunk width (elements of the flattened (k s) column),
    ow: output-feature width kept resident in SBUF at a time.
    """
    import concourse.bacc as bacc
    import concourse.mybir as mybir
    import concourse.tile as tile

    dt = mybir.dt
    Alu = mybir.AluOpType

    kt = d // 128        # contraction tiles
    ms_n = s // 128      # token blocks
    oh_n = o // ow       # weight-resident passes
    nj = ow // 512       # 512-wide matmuls per k per pass
    colw = kt * 128      # flattened quant-column width (== d)
    qc_n = colw // ch    # quant chunks per token block

    nc = bacc.Bacc("TRN2", target_bir_lowering=False, debug=False)

    xT = nc.dram_tensor("xT", [d, s], dt.float32, kind="ExternalInput")
    w = nc.dram_tensor("w", [d, o], dt.bfloat16, kind="ExternalInput")
    bb = nc.dram_tensor("bb", [128, o], dt.bfloat16, kind="ExternalInput")
    out = nc.dram_tensor("out", [s, o], dt.float32, kind="ExternalOutput")

    with tile.TileContext(nc) as tc:
        with (
            tc.tile_pool(name="const", bufs=1) as constp,
            tc.tile_pool(name="wt", bufs=1) as wtp,
            tc.tile_pool(name="xq", bufs=3) as xqp,
            tc.tile_pool(name="xf", bufs=4) as xfp,
            tc.tile_pool(name="vv", bufs=4) as vp,
            tc.tile_pool(name="m2", bufs=2) as m2p,
            tc.tile_pool(name="bt", bufs=2) as btp,
            tc.tile_pool(name="osb", bufs=2 * nj) as osbp,
            tc.tile_pool(name="ps", bufs=2 * nj, space="PSUM") as psp,
        ):
            biasb = constp.tile([128, o], dt.bfloat16)

            def emit_quant(ms):
                """Quantize the ms-th 128-token column (all of d) into a
                fresh xq tile; returns the tile."""
                xq = xqp.tile([128, colw], dt.bfloat16, tag="xq", name="xq")
                xsrc = xT[:, ms * 128 : (ms + 1) * 128].rearrange(
                    "(k p) t -> p k t", p=128
                )
                kpc = ch // 128  # k-slabs per quant chunk
                for q in range(qc_n):
                    xf = xfp.tile([128, ch], dt.float32, tag="xf", name="xf")
                    nc.sync.dma_start(
                        xf[:].rearrange("p (k t) -> p k t", t=128),
                        xsrc[:, q * kpc : (q + 1) * kpc, :],
                    )
                    # S0_bits = bits(f32(u & 0xFF800000) * 2) & 0xFF000000
                    # (walrus forbids mixing arith+bitwise in one op, and
                    # int arithmetic may run through the fp32 datapath, so
                    # keep each instruction in a single domain)
                    qeng = nc.gpsimd if QUANT_S_ON_GPSIMD else nc.vector
                    a = vp.tile([128, ch], dt.int32, tag="vv", name="a")
                    qeng.tensor_scalar(
                        a[:], xf[:].bitcast(dt.int32), -8388608, None,
                        Alu.bitwise_and,
                    )
                    m2 = m2p.tile([128, ch], dt.float32, tag="m2", name="m2")
                    qeng.tensor_scalar(
                        m2[:], a[:].bitcast(dt.float32), 2.0, None, Alu.mult
                    )
                    v = vp.tile([128, ch], dt.int32, tag="vv", name="v")
                    qeng.tensor_scalar(
                        v[:], m2[:].bitcast(dt.int32), -16777216, None,
                        Alu.bitwise_and,
                    )
                    bt = btp.tile([128, ch], dt.float32, tag="bt", name="bt")
                    # b = S0*65536 + x   (S = S0*65536 is the rounding magnet)
                    nc.vector.scalar_tensor_tensor(
                        bt[:], v[:].bitcast(dt.float32), 65536.0, xf[:],
                        Alu.mult, Alu.add,
                    )
                    # xq = b - S  (exact, <=8 significand bits -> bf16 exact)
                    nc.vector.scalar_tensor_tensor(
                        xq[:, q * ch : (q + 1) * ch],
                        v[:].bitcast(dt.float32), -65536.0, bt[:],
                        Alu.mult, Alu.add,
                    )
                return xq

            def emit_wt(oh, k, half=None):
                """DMA one weight slab.  half=None: whole [128, ow] slab in
                one transfer (allocates the tile).  half=(tile, 0|1): fill
                only that half of an existing tile (used to defer the high
                half of pass 0 past the ramp; MMs on each half depend only
                on its own DMA)."""
                if half is None:
                    wtile = wtp.tile(
                        [128, ow], dt.bfloat16, tag=f"w{k}", name=f"wtile{k}"
                    )
                    nc.sync.dma_start(
                        wtile[:],
                        w[k * 128 : (k + 1) * 128, oh * ow : (oh + 1) * ow],
                    )
                    return wtile
                wtile, h = half
                hw_ = ow // 2
                nc.sync.dma_start(
                    wtile[:, h * hw_ : (h + 1) * hw_],
                    w[
                        k * 128 : (k + 1) * 128,
                        oh * ow + h * hw_ : oh * ow + (h + 1) * hw_,
                    ],
                )
                return wtile

            def emit_group(oh, ms, js, xq, wt):
                """One accumulation group: output block out[ms, oh*ow + js
                512-columns], full contraction over kt k-slabs."""
                ps = [
                    psp.tile([128, 512], dt.float32, tag="ps", name="ps")
                    for _ in js
                ]
                for k in range(kt):
                    lhsT = xq[:, k * 128 : (k + 1) * 128]
                    for i, j in enumerate(js):
                        nc.tensor.matmul(
                            ps[i][:],
                            lhsT,
                            wt[k][:, j * 512 : (j + 1) * 512],
                            start=(k == 0),
                            stop=(k == kt - 1),
                        )
                for i, j in enumerate(js):
                    osb = osbp.tile([128, 512], dt.float32, tag="osb")
                    nc.vector.tensor_add(
                        osb[:],
                        ps[i][:],
                        biasb[:, oh * ow + j * 512 : oh * ow + (j + 1) * 512],
                    )
                    nc.sync.dma_start(
                        out[
                            ms * 128 : (ms + 1) * 128,
                            oh * ow + j * 512 : oh * ow + (j + 1) * 512,
                        ],
                        osb[:],
                    )

            all_js = tuple(range(nj))
            lo_js = tuple(range(nj // 2)) or all_js
            hi_js = tuple(range(nj // 2, nj))
            for oh in range(oh_n):
                split_ramp = oh == 0 and ms_n > 1 and nj >= 2
                if split_ramp:
                    # Ramp is DMA-bound on the first column's weights: feed
                    # the PE the low output-half (half the weight bytes)
                    # for columns 0-1 first; the high half streams during
                    # that compute.
                    wt = [
                        wtp.tile(
                            [128, ow], dt.bfloat16, tag=f"w{k}", name=f"wtile{k}"
                        )
                        for k in range(kt)
                    ]
                    emit_wt(oh, 0, (wt[0], 0))
                    xq0 = emit_quant(0)
                    nc.sync.dma_start(biasb[:], bb[:])
                    for k in range(1, kt):
                        emit_wt(oh, k, (wt[k], 0))
                    xq1 = emit_quant(1)
                    for k in range(kt):
                        emit_wt(oh, k, (wt[k], 1))
                    groups = (
                        [(0, lo_js), (1, lo_js), (0, hi_js), (1, hi_js)]
                        + [(m, all_js) for m in range(2, ms_n)]
                    )
                    col_tiles = {0: xq0, 1: xq1}
                else:
                    wt = [emit_wt(oh, 0)]
                    xq0 = emit_quant(0)
                    if oh == 0:
                        nc.sync.dma_start(biasb[:], bb[:])
                    col_tiles = {0: xq0}
                    if ms_n > 1:
                        col_tiles[1] = emit_quant(1)
                    for k in range(1, kt):
                        wt.append(emit_wt(oh, k))
                    groups = [(m, all_js) for m in range(ms_n)]

                # columns quantized two ahead of their first (full) use;
                # during the split ramp, hold prefetch back so the weight
                # high-half DMAs aren't stuck behind extra x chunks
                for idx, (ms, js) in enumerate(groups):
                    if ms not in col_tiles:
                        col_tiles[ms] = emit_quant(ms)
                    if not (split_ramp and idx < 2):
                        nxt = max(col_tiles) + 1
                        if nxt < ms_n and nxt <= ms + 2:
                            col_tiles[nxt] = emit_quant(nxt)
                    emit_group(oh, ms, js, col_tiles[ms], wt)
    nc.compile()
    _dedupe_ldweights(nc)
    return nc


def _dedupe_ldweights(nc):
    """Drop redundant InstLdweights: walrus emits LDWEIGHTS before every
    MATMUL, but consecutive matmuls sharing the same stationary operand only
    need the first load (the PE weight buffer persists across matmuls).
    Only sync-free LDWs with a byte-identical weights access pattern to the
    previous (surviving) LDW are dropped; any other instruction class in
    between resets the tracked state."""

    def sig(ins):
        ap = ins.ins[-1]
        try:
            return (
                ap.memref if hasattr(ap, "memref") else None,
                ap.offset,
                str(ap.ap),
                str(getattr(ap, "dtype", None)),
            )
        except Exception:
            return None

    dropped = 0
    for fn in nc.m.functions:
        for blk in fn.blocks:
            last = None
            keep = []
            for ins in blk.instructions:
                nm = type(ins).__name__
                if nm == "InstLdweights":
                    si = ins.sync_info
                    clean = si is None or (
                        len(si.on_wait) == 0 and len(si.on_update) == 0
                    )
                    s = sig(ins)
                    if clean and s is not None and s == last:
                        dropped += 1
                        continue  # redundant reload of identical weights
                    last = s if s is not None else None
                    keep.append(ins)
                elif nm in ("InstMatmult", "InstEventSemaphore", "InstNop"):
                    keep.append(ins)
                else:
                    last = None
                    keep.append(ins)
            if len(keep) != len(blk.instructions):
                blk.instructions[:] = keep
    return dropped


def _prep_inputs(x, weight, bias):
    import ml_dtypes

    x = np.ascontiguousarray(np.asarray(x, dtype=np.float32))
    weight = np.asarray(weight, dtype=np.float32)
    bias = np.asarray(bias, dtype=np.float32)

    # per-core transposed x: (B, D, S)
    xT = np.ascontiguousarray(x.transpose(0, 2, 1))
    # weight.T in bf16 (exact: AF16 subset of bf16), contiguous (D, O)
    wT = np.ascontiguousarray(weight.T).astype(ml_dtypes.bfloat16)
    bb = np.ascontiguousarray(
        np.broadcast_to(bias.reshape(1, -1), (128, bias.shape[0]))
    ).astype(ml_dtypes.bfloat16)
    return xT, wT, bb


def kernel(x, weight, bias, _trace=False, _trace_kwargs=None):
    from concourse import bass_utils

    xT, wT, bb = _prep_inputs(x, weight, bias)
    nc = build_nc()
    in_maps = [{"xT": xT[c], "w": wT, "bb": bb} for c in range(N_CORES)]
    res = bass_utils.run_bass_kernel_spmd(
        nc,
        in_maps,
        core_ids=list(range(N_CORES)),
        trace=_trace,
        **(_trace_kwargs or {}),
    )
    outs = np.stack([res.results[c]["out"] for c in range(N_CORES)])
    if _trace:
        kernel.last_results = res
    return outs.astype(np.float32, copy=False)


if __name__ == "__main__":
    rng = np.random.default_rng(0)
    x = rng.standard_normal((B, S, D), dtype=np.float32)
    weight = rng.standard_normal((O, D), dtype=np.float32) * 0.02
    bias = rng.standard_normal((O,), dtype=np.float32) * 0.02
    out = kernel(x, weight, bias)
    print(out.shape, out.dtype)
